# revision 16
# baseline (speedup 1.0000x reference)
"""Trainium2 Bass kernel for nn_CantorMultiheadFusionV2.

Math note: the Cantor-KNN fusion geometry is input-independent and fully
saturated at float32 — every row's inverse-distance softmax weight is
exactly one-hot on the row itself (self-distance 0 gives logit 1e8 while
every competitor logit is at most ~1/4.3e-7, so exp(logit - 1e8)
underflows to 0.0 in float32). The neighbor-fusion stage is therefore
bit-exactly the identity, and the whole module reduces to

    out = x + (x @ W_in + b_in) @ W_out + b_out

which this kernel computes as a residual two-matmul MLP, data-parallel
over the 4096 (B*S) rows across 8 NeuronCores (512 rows per core;
weights replicated per the sharding hint).
"""

import os
import sys

import numpy as np

for _p in ("/opt/trn_rl_repo", "/root/.axon_site/_ro/trn_rl_repo"):
    if os.path.isdir(_p) and _p not in sys.path:
        sys.path.insert(0, _p)

import concourse.bass as bass
import concourse.mybir as mybir
from concourse.bass_utils import run_bass_kernel_spmd
from concourse.masks import make_identity
from concourse.tile import TileContext
from concourse.tile_rust import add_dep_helper

N_CORES = 8
B, S, D = 2, 2048, 512
ROWS = (B * S) // N_CORES  # 512 rows per core
P = 128
MT = ROWS // P  # 4 row tiles per core
KT = D // P     # 4 contraction tiles
FP = mybir.dt.float32

LAST_EXEC_NS = None


def _build(with_bias: bool) -> bass.Bass:
    nc = bass.Bass()

    x_in = nc.declare_dram_parameter("x", [ROWS, D], FP, isOutput=False)
    w_in = nc.declare_dram_parameter("w_in", [D, D], FP, isOutput=False)
    w_out = nc.declare_dram_parameter("w_out", [D, D], FP, isOutput=False)
    if with_bias:
        b_in = nc.declare_dram_parameter("b_in", [1, D], FP, isOutput=False)
        b_out = nc.declare_dram_parameter("b_out", [1, D], FP, isOutput=False)
    y_out = nc.declare_dram_parameter("y", [ROWS, D], FP, isOutput=True)

    # Grouped [128, 4, 512] views so each tensor moves as one 1MB DMA.
    xg = x_in[:].rearrange("(m p) d -> p m d", p=P)
    wig = w_in[:].rearrange("(k p) d -> p k d", p=P)
    wog = w_out[:].rearrange("(k p) d -> p k d", p=P)
    yg = y_out[:].rearrange("(m p) d -> p m d", p=P)

    with TileContext(nc) as tc:
        with (
            tc.tile_pool(name="const", bufs=1) as const_pool,
            tc.tile_pool(name="big", bufs=1) as big_pool,
            tc.tile_pool(name="xt", bufs=1) as xt_pool,
            tc.tile_pool(name="ht", bufs=1) as ht_pool,
            tc.tile_pool(name="out", bufs=1) as out_pool,
            tc.tile_pool(name="tp_ps", bufs=4, space="PSUM") as tp_psum,
            tc.tile_pool(name="h_ps", bufs=2, space="PSUM") as h_psum,
            tc.tile_pool(name="o_ps", bufs=2, space="PSUM") as o_psum,
        ):
            # --- loads (one 1MB DMA each) ---
            x_t = big_pool.tile([P, MT, D], FP, tag="x_t")
            wi_t = big_pool.tile([P, KT, D], FP, tag="wi_t")
            wo_t = big_pool.tile([P, KT, D], FP, tag="wo_t")
            nc.sync.dma_start(out=x_t[:], in_=xg)
            nc.sync.dma_start(out=wi_t[:], in_=wig)
            nc.sync.dma_start(out=wo_t[:], in_=wog)

            ident = const_pool.tile([P, P], FP, tag="ident")
            make_identity(nc, ident[:])
            dve_scratch = const_pool.tile([1, 1], FP, tag="dve_scratch")

            if with_bias:
                bi_t = const_pool.tile([1, D], FP, tag="bi_t")
                bo_t = const_pool.tile([1, D], FP, tag="bo_t")
                ones = const_pool.tile([1, max(ROWS, D)], FP, tag="ones")
                nc.sync.dma_start(out=bi_t[:], in_=b_in[:])
                nc.sync.dma_start(out=bo_t[:], in_=b_out[:])
                nc.gpsimd.memset(ones[:], 1.0)

            # Walrus codegen allows at most ONE semaphore wait per Matmult.
            # Each stage therefore gets a [1,1] "absorber" matmul that takes
            # one of the stage's two producer semaphores into PE's observed
            # clock; explicit same-engine ordering edges force every real
            # matmul of the stage after its absorber, leaving each real
            # matmul with at most one new wait. Absorbers gate only on data
            # that stage needs anyway, so DMA/compute overlap is preserved.
            def pe_absorb(src_ap):
                # Standalone LDWEIGHTS (bf16 view — fp32 standalone LDW is
                # rejected by walrus) reads the tensor so the producer's
                # semaphore lands here, without any PSUM write whose drain
                # tracking would leak onto later matmul groups.
                return nc.tensor.ldweights(
                    src_ap.bitcast(mybir.dt.bfloat16)
                ).ins

            # --- build xT tiles: xt[d][p, s] = x[s, d*128+p] ---
            # All 4 row-block transposes for one d-block land in a single
            # PSUM tile so each xt tile has exactly one writer (the DVE
            # copy) — keeps the sync-wait fan-in on consumers at 1.
            xt = []
            abs_ident = None
            for d in range(KT):
                pt = tp_psum.tile([P, ROWS], FP, tag="tp")
                if abs_ident is None:
                    # absorbs the gpsimd (identity) semaphore
                    abs_ident = pe_absorb(ident[:1, :1])
                for m in range(MT):
                    ti = nc.tensor.transpose(
                        pt[:, m * P : (m + 1) * P],
                        x_t[:, m, d * P : (d + 1) * P],
                        ident[:],
                    )
                    add_dep_helper(ti.ins, abs_ident, sync=False, reason="pe-wait-cap")
                xt_d = xt_pool.tile([P, ROWS], FP, tag=f"xt{d}")
                nc.vector.tensor_copy(out=xt_d[:], in_=pt[:])
                xt.append(xt_d)

            # --- mm1: ht[j][p, s] = sum_d W_in[d, j*128+p] * xT[d, s] (+ b_in) ---
            ht = []
            abs_wi = None
            abs_ones = None
            for j in range(KT):
                ph = h_psum.tile([P, ROWS], FP, tag="ph")
                if j >= 2:
                    # Full-tile DVE claim of the reused PSUM bank: the PE
                    # drain + reader-release waits land on this DVE write,
                    # so the next matmul's WAW dep is the claim alone.
                    nc.vector.memset(ph[:], 0.0)
                if abs_wi is None:
                    # absorbs the W_in DMA semaphore
                    abs_wi = pe_absorb(wi_t[:1, 0, :1])
                    if with_bias:
                        # absorbs the gpsimd tick of the ones-memset
                        abs_ones = pe_absorb(ones[:1, :1])
                for k in range(KT):
                    mi = nc.tensor.matmul(
                        ph[:],
                        wi_t[:, k, j * P : (j + 1) * P],
                        xt[k][:],
                        start=(k == 0),
                        stop=(k == KT - 1) and not with_bias,
                    )
                    add_dep_helper(mi.ins, abs_wi, sync=False, reason="pe-wait-cap")
                if with_bias:
                    # rank-1 broadcast: += b_in[j*128+p] * ones[s]
                    mi = nc.tensor.matmul(
                        ph[:],
                        bi_t[:1, j * P : (j + 1) * P],
                        ones[:1, :ROWS],
                        start=False,
                        stop=True,
                    )
                    add_dep_helper(mi.ins, abs_ones, sync=False, reason="pe-wait-cap")
                ht_j = ht_pool.tile([P, ROWS], FP, tag=f"ht{j}")
                nc.vector.tensor_copy(out=ht_j[:], in_=ph[:])
                ht.append(ht_j)

            # --- mm2 + residual: y[m*128+p, :] = sum_j ht[j][:, m-blk].T @ W_out[j-blk, :] (+ b_out) + x ---
            abs_wo = None
            abs_xr = None
            out_t = out_pool.tile([P, MT, D], FP, tag="out_t")
            for m in range(MT):
                po = o_psum.tile([P, D], FP, tag="po")
                if m >= 2:
                    nc.vector.memset(po[:], 0.0)
                if abs_wo is None:
                    # absorbs the W_out DMA semaphore
                    abs_wo = pe_absorb(wo_t[:1, 0, :1])
                    # absorbs the x DMA semaphore on the DVE side for the
                    # residual adds (psum wait + x wait would be 2 otherwise)
                    abs_xr = nc.vector.tensor_copy(
                        out=dve_scratch[:1, :1], in_=x_t[:1, 0, :1]
                    ).ins
                for j in range(KT):
                    mi = nc.tensor.matmul(
                        po[:],
                        ht[j][:, m * P : (m + 1) * P],
                        wo_t[:, j, :],
                        start=(j == 0),
                        stop=(j == KT - 1) and not with_bias,
                    )
                    add_dep_helper(mi.ins, abs_wo, sync=False, reason="pe-wait-cap")
                if with_bias:
                    # rank-1 broadcast: += ones[s] * b_out[d]
                    mi = nc.tensor.matmul(
                        po[:],
                        ones[:1, m * P : (m + 1) * P],
                        bo_t[:1, :],
                        start=False,
                        stop=True,
                    )
                    add_dep_helper(mi.ins, abs_ones, sync=False, reason="pe-wait-cap")
                ai = nc.vector.tensor_add(
                    out=out_t[:, m, :], in0=po[:], in1=x_t[:, m, :]
                )
                add_dep_helper(ai.ins, abs_xr, sync=False, reason="dve-wait-cap")
            # single grouped store keeps the DMA-lane count (and with it the
            # kernel-tail drain's wait list) small
            nc.sync.dma_start(out=yg, in_=out_t[:])

    return nc


# Per-opcode sync-wait capacity of walrus codegen on this toolchain
# (hardware TPB EVENTS struct has a single wait slot; walrus accepts 2 on
# DVE/ACT compound ops but only 1 on Matmult and CTRL_NO-lowered ops).
_WAIT_CAPS = {"Matmult": 1, "Ldweights": 1, "Drain": 1, "NoOp": 1}
_WAIT_CAP_DEFAULT = 2


def _legalize_waits(nc: bass.Bass) -> None:
    """Split instructions whose sync-wait list exceeds walrus's per-opcode
    capacity: excess waits move onto freshly inserted same-engine NOPs
    directly before the instruction (engines execute their stream in order,
    so a preceding NOP carrying the wait is semantically identical)."""
    for fn in nc.m.functions:
        for bb in fn.blocks:
            insts = bb.instructions
            out = []
            changed = False
            for inst in insts:
                si = inst.sync_info
                waits = list(si.on_wait) if si is not None else []
                cap = _WAIT_CAPS.get(getattr(inst, "opcode", ""), _WAIT_CAP_DEFAULT)
                if len(waits) > cap:
                    keep = waits[:cap]
                    excess = waits[cap:]
                    for w in excess:
                        nop = mybir.InstNoOp(
                            name=nc.get_next_instruction_name(),
                            engine=inst.engine,
                            sync_info=mybir.SyncInfo(on_wait=[w], on_update=[]),
                            bass_nofuse=True,
                        )
                        out.append(nop)
                    inst.sync_info = mybir.SyncInfo(
                        on_wait=keep, on_update=list(si.on_update)
                    )
                    changed = True
                out.append(inst)
            if changed:
                bb.instructions = out


_NC_CACHE: dict = {}
_EXEC_CACHE: dict = {}


class _Executor:
    """Cached jitted SPMD executor (mirrors bass2jax.run_bass_via_pjrt's
    multi-core path) so repeated kernel() calls reuse one compiled NEFF."""

    def __init__(self, nc: bass.Bass):
        import jax
        import jax.numpy as jnp
        from jax.experimental.shard_map import shard_map
        from jax.sharding import Mesh, PartitionSpec
        from concourse import bass2jax

        bass2jax.install_neuronx_cc_hook()
        self.nc = nc
        assert nc.dbg_addr is None
        partition_name = (
            nc.partition_id_tensor.name if nc.partition_id_tensor else None
        )

        in_names: list[str] = []
        out_names: list[str] = []
        out_avals = []
        zero_outs: list[np.ndarray] = []
        for alloc in nc.m.functions[0].allocations:
            if not isinstance(alloc, mybir.MemoryLocationSet):
                continue
            name = alloc.memorylocations[0].name
            if alloc.kind == "ExternalInput":
                if name != partition_name:
                    in_names.append(name)
            elif alloc.kind == "ExternalOutput":
                out_names.append(name)
                shape = tuple(alloc.tensor_shape)
                dtype = mybir.dt.np(alloc.dtype)
                out_avals.append(jax.core.ShapedArray(shape, dtype))
                zero_outs.append(np.zeros(shape, dtype))
        self.in_names = list(in_names)
        self.out_names = out_names
        self.zero_outs = zero_outs
        all_in_names = in_names + out_names
        if partition_name is not None:
            all_in_names = all_in_names + [partition_name]

        def _body(*args):
            operands = list(args)
            if partition_name is not None:
                operands.append(bass2jax.partition_id_tensor())
            outs = bass2jax._bass_exec_p.bind(
                *operands,
                out_avals=tuple(out_avals),
                in_names=tuple(all_in_names),
                out_names=tuple(out_names),
                lowering_input_output_aliases=(),
                sim_require_finite=True,
                sim_require_nnan=True,
                nc=nc,
            )
            return tuple(outs)

        devices = jax.devices()[:N_CORES]
        self.mesh = Mesh(np.asarray(devices), ("core",))
        n_args = len(in_names) + len(out_names)
        self.jitted = jax.jit(
            shard_map(
                _body,
                mesh=self.mesh,
                in_specs=(PartitionSpec("core"),) * n_args,
                out_specs=(PartitionSpec("core"),) * len(out_names),
                check_rep=False,
            )
        )

    def run(self, per_core_inputs: dict[str, list[np.ndarray]]):
        concat = [
            np.concatenate(per_core_inputs[name], axis=0) for name in self.in_names
        ] + [
            np.concatenate([z] * N_CORES, axis=0) for z in self.zero_outs
        ]
        outs = self.jitted(*concat)
        return {
            name: np.asarray(outs[i]) for i, name in enumerate(self.out_names)
        }


def _get_executor(with_bias: bool) -> _Executor:
    if with_bias not in _EXEC_CACHE:
        if with_bias not in _NC_CACHE:
            nc = _build(with_bias)
            _legalize_waits(nc)
            _NC_CACHE[with_bias] = nc
        _EXEC_CACHE[with_bias] = _Executor(_NC_CACHE[with_bias])
    return _EXEC_CACHE[with_bias]


def _make_per_core_inputs(x, W_in, b_in, W_out, b_out, with_bias):
    xf = x.reshape(B * S, D)
    per_core = {
        "x": [np.ascontiguousarray(xf[c * ROWS : (c + 1) * ROWS]) for c in range(N_CORES)],
        "w_in": [W_in] * N_CORES,
        "w_out": [W_out] * N_CORES,
    }
    if with_bias:
        per_core["b_in"] = [b_in] * N_CORES
        per_core["b_out"] = [b_out] * N_CORES
    return per_core


def kernel(x, W_in, b_in, W_out, b_out):
    x = np.ascontiguousarray(np.asarray(x, dtype=np.float32))
    W_in = np.ascontiguousarray(np.asarray(W_in, dtype=np.float32))
    W_out = np.ascontiguousarray(np.asarray(W_out, dtype=np.float32))
    b_in = np.ascontiguousarray(np.asarray(b_in, dtype=np.float32)).reshape(1, D)
    b_out = np.ascontiguousarray(np.asarray(b_out, dtype=np.float32)).reshape(1, D)

    with_bias = bool(b_in.any() or b_out.any())
    ex = _get_executor(with_bias)
    outs = ex.run(_make_per_core_inputs(x, W_in, b_in, W_out, b_out, with_bias))
    return outs["y"].reshape(B, S, D).astype(np.float32)


def bench(x, W_in, b_in, W_out, b_out, iters: int = 20):
    """Steady-state timing: device-resident inputs, repeated dispatch of the
    cached executable; returns (min_seconds, all_times). Includes axon
    dispatch overhead, so treat as an upper bound on HW kernel time."""
    import time
    import jax

    x = np.ascontiguousarray(np.asarray(x, dtype=np.float32))
    W_in = np.ascontiguousarray(np.asarray(W_in, dtype=np.float32))
    W_out = np.ascontiguousarray(np.asarray(W_out, dtype=np.float32))
    b_in = np.ascontiguousarray(np.asarray(b_in, dtype=np.float32)).reshape(1, D)
    b_out = np.ascontiguousarray(np.asarray(b_out, dtype=np.float32)).reshape(1, D)
    with_bias = bool(b_in.any() or b_out.any())
    ex = _get_executor(with_bias)
    per_core = _make_per_core_inputs(x, W_in, b_in, W_out, b_out, with_bias)

    from jax.sharding import NamedSharding, PartitionSpec

    sh = NamedSharding(ex.mesh, PartitionSpec("core"))
    concat = [
        jax.device_put(np.concatenate(per_core[name], axis=0), sh)
        for name in ex.in_names
    ] + [
        jax.device_put(np.concatenate([z] * N_CORES, axis=0), sh)
        for z in ex.zero_outs
    ]
    # warmup (compile + first run)
    outs = ex.jitted(*concat)
    jax.block_until_ready(outs)
    times = []
    for _ in range(iters):
        t0 = time.perf_counter()
        outs = ex.jitted(*concat)
        jax.block_until_ready(outs)
        times.append(time.perf_counter() - t0)
    return min(times), times
